# revision 8
# baseline (speedup 1.0000x reference)
"""Trainium2 Bass kernel: batched multi-head attention with padded KV.

Problem shape (hardcoded): qkv [128, 64, 32, 384] f32 packed Q|K|V on the
last axis, head_dim 128, kv_seq_len scalar (<= 64). Output [128, 64, 32, 128].

Sharding: data-parallel over the request (batch) axis across 8 NeuronCores
(16 requests per core). Each core runs the same SPMD program on its slice.

Per-core pipeline, per pair of heads (2 requests stacked on partitions):
  DMA qkv chunk -> cast f32->bf16 (gpsimd) -> PE transpose Q,K (d onto
  partitions) -> psum->sbuf copy (DVE) -> scores matmul (PE) -> exp with
  accumulated denominator (Act) -> reciprocal (DVE) -> PE transpose of the
  exp matrix -> AV matmul (PE) -> normalize-on-copy psum->sbuf (Act) -> DMA.
"""

from contextlib import ExitStack

import numpy as np

import bass_rust
import concourse.bass as bass
import concourse.mybir as mybir
import concourse.tile as tile
from concourse.bass_utils import run_bass_kernel_spmd
from concourse.masks import make_identity

NUM_REQ = 128
SEQ = 64
NUM_HEAD = 32
HEAD_DIM = 128
N_CORES = 8
B_CORE = NUM_REQ // N_CORES  # 16 requests per core
N_BLK = B_CORE // 2          # 8 two-request blocks
H_CHUNK = 8                  # heads per DMA chunk
N_CHUNK = NUM_HEAD // H_CHUNK
SCALE = 1.0 / float(np.sqrt(HEAD_DIM))

DT = mybir.dt
F32 = DT.float32
BF16 = DT.bfloat16

_BUILD_CACHE: dict[int, bass.Bass] = {}


def _legalize_waits(nc: bass.Bass, cap_default: int = 1, cap_ev: int = 2) -> int:
    """Walrus codegen accepts at most 1 sync wait per engine instruction
    (2 on InstEventSemaphore). Tile's scheduler attaches more; spill the
    excess into dedicated InstEventSemaphore instructions placed right
    before the owning instruction on the same engine — the engine stream
    is in-order, so blocking at the preceding instruction is equivalent."""
    ctr = 0
    for func in nc.m.functions:
        for blk in func.blocks:
            out = []
            changed = False
            for inst in blk.instructions:
                si = inst.sync_info
                cap = (
                    cap_ev
                    if isinstance(inst, mybir.InstEventSemaphore)
                    else cap_default
                )
                if si is not None:
                    waits = list(si.on_wait)
                    if len(waits) > cap:
                        extra, keep = waits[:-cap], waits[-cap:]
                        for j in range(0, len(extra), 2):
                            ev = mybir.InstEventSemaphore(
                                name=f"I-evw{ctr}", ins=[], outs=[]
                            )
                            ctr += 1
                            ev.engine = inst.engine
                            ev.sync_info = bass_rust.SyncInfo(
                                on_wait=extra[j : j + 2], on_update=[]
                            )
                            out.append(ev)
                        si.on_wait = keep
                        changed = True
                out.append(inst)
            if changed:
                blk.instructions = out
    return ctr


def _build(L: int) -> bass.Bass:
    """Build the per-core SPMD program for active kv length L (1..64)."""
    nc = bass.Bass()
    qkv = nc.declare_dram_parameter(
        "qkv", [B_CORE, SEQ, NUM_HEAD, 3 * HEAD_DIM], F32, isOutput=False
    )
    out = nc.declare_dram_parameter(
        "out", [B_CORE, SEQ, NUM_HEAD, HEAD_DIM], F32, isOutput=True
    )

    with tile.TileContext(nc) as tc:
        with ExitStack() as ctx:
            singles = ctx.enter_context(tc.tile_pool(name="singles", bufs=1))
            pool_in = ctx.enter_context(tc.tile_pool(name="in", bufs=2))
            pool_bf = ctx.enter_context(tc.tile_pool(name="bf", bufs=2))
            pool_qtkt = ctx.enter_context(tc.tile_pool(name="qtkt", bufs=3))
            pool_p = ctx.enter_context(tc.tile_pool(name="p", bufs=3))
            pool_pt = ctx.enter_context(tc.tile_pool(name="pt", bufs=3))
            pool_sm = ctx.enter_context(tc.tile_pool(name="sm", bufs=4))
            pool_out = ctx.enter_context(tc.tile_pool(name="out", bufs=3))
            ps_qtkt = ctx.enter_context(
                tc.tile_pool(name="ps_qtkt", bufs=2, space="PSUM")
            )
            ps_sc = ctx.enter_context(tc.tile_pool(name="ps_sc", bufs=2, space="PSUM"))
            ps_pt = ctx.enter_context(tc.tile_pool(name="ps_pt", bufs=2, space="PSUM"))
            ps_av = ctx.enter_context(tc.tile_pool(name="ps_av", bufs=2, space="PSUM"))

            ident = singles.tile([128, 128], BF16)
            make_identity(nc, ident)

            D = HEAD_DIM
            for j in range(N_BLK):
                for c in range(N_CHUNK):
                    chunk = pool_in.tile([128, H_CHUNK * 3 * D], F32)
                    src = qkv[2 * j : 2 * j + 2, :, c * H_CHUNK : (c + 1) * H_CHUNK, :]
                    nc.gpsimd.dma_start(
                        out=chunk, in_=src.rearrange("b s h d -> (b s) (h d)")
                    )
                    chbf = pool_bf.tile([128, H_CHUNK * 3 * D], BF16)
                    nc.gpsimd.tensor_copy(chbf[:, :], chunk[:, :])

                    for g in range(H_CHUNK // 4):  # groups of 4 heads -> one out DMA
                        out4 = pool_out.tile([128, 4 * D], F32)
                        av4 = ps_av.tile([128, 4 * D], F32)
                        for pi in range(2):  # pairs of heads within the group
                            ha = 4 * g + 2 * pi  # head index within chunk
                            ca = ha * 3 * D      # column base of head a in chunk
                            cb = (ha + 1) * 3 * D
                            # --- transpose Q,K of both heads: d onto partitions
                            qtkt_ps = ps_qtkt.tile([128, 512], BF16)
                            nc.tensor.transpose(
                                qtkt_ps[:, 0:128], chbf[:, ca : ca + D], ident[:, :]
                            )
                            nc.tensor.transpose(
                                qtkt_ps[:, 128:256],
                                chbf[:, ca + D : ca + 2 * D],
                                ident[:, :],
                            )
                            nc.tensor.transpose(
                                qtkt_ps[:, 256:384], chbf[:, cb : cb + D], ident[:, :]
                            )
                            nc.tensor.transpose(
                                qtkt_ps[:, 384:512],
                                chbf[:, cb + D : cb + 2 * D],
                                ident[:, :],
                            )
                            qtkt = pool_qtkt.tile([128, 512], BF16)
                            nc.vector.tensor_copy(qtkt[:, :], qtkt_ps[:, :])

                            # --- scores: [q, k] per (head, request)
                            # layout in sc2: partitions q-cat (b0|b1), free 0:64
                            # head a, 64:128 head b
                            sc2 = ps_sc.tile([128, 128], F32)
                            nc.tensor.matmul(
                                sc2[0:64, 0:L],
                                qtkt[:, 0:64],
                                qtkt[:, 128 : 128 + L],
                                start=True,
                                stop=True,
                            )
                            nc.tensor.matmul(
                                sc2[64:128, 0:L],
                                qtkt[:, 64:128],
                                qtkt[:, 192 : 192 + L],
                                start=True,
                                stop=True,
                            )
                            nc.tensor.matmul(
                                sc2[0:64, 64 : 64 + L],
                                qtkt[:, 256:320],
                                qtkt[:, 384 : 384 + L],
                                start=True,
                                stop=True,
                            )
                            nc.tensor.matmul(
                                sc2[64:128, 64 : 64 + L],
                                qtkt[:, 320:384],
                                qtkt[:, 448 : 448 + L],
                                start=True,
                                stop=True,
                            )

                            # --- exp(scale * s), accumulating the denominator
                            p2 = pool_p.tile([128, 128], BF16)
                            den2 = pool_sm.tile([128, 2], F32)
                            nc.scalar.activation(
                                p2[:, 0:L],
                                sc2[:, 0:L],
                                mybir.ActivationFunctionType.Exp,
                                bias=0.0,
                                scale=SCALE,
                                accum_out=den2[:, 0:1],
                            )
                            nc.scalar.activation(
                                p2[:, 64 : 64 + L],
                                sc2[:, 64 : 64 + L],
                                mybir.ActivationFunctionType.Exp,
                                bias=0.0,
                                scale=SCALE,
                                accum_out=den2[:, 1:2],
                            )
                            rec2 = pool_sm.tile([128, 2], F32)
                            nc.vector.reciprocal(rec2[:, :], den2[:, :])

                            # --- transpose p: k onto partitions
                            pt_ps = ps_pt.tile([64, 256], BF16)
                            nc.tensor.transpose(
                                pt_ps[0:L, 0:128], p2[:, 0:L], ident[:, :]
                            )
                            nc.tensor.transpose(
                                pt_ps[0:L, 128:256], p2[:, 64 : 64 + L], ident[:, :]
                            )
                            pt2 = pool_pt.tile([128, 256], BF16)
                            src_b0 = pt_ps[0:L].rearrange("p (t q) -> p t q", t=2)
                            dst_b0 = pt2[0:L].rearrange("p (t q) -> p t q", t=2)
                            nc.vector.tensor_copy(
                                dst_b0[:, :, 0:64], src_b0[:, :, 0:64]
                            )
                            src_b1 = pt_ps[0:L].rearrange("p (t q) -> p t q", t=2)
                            dst_b1 = pt2[64 : 64 + L].rearrange(
                                "p (t q) -> p t q", t=2
                            )
                            nc.vector.tensor_copy(
                                dst_b1[:, :, 64:128], src_b1[:, :, 64:128]
                            )

                            # --- attn @ V, unnormalized, into the group psum
                            sa = 2 * pi * D
                            sb = sa + D
                            va = ca + 2 * D
                            vb = cb + 2 * D
                            nc.tensor.matmul(
                                av4[0:64, sa : sa + D],
                                pt2[0:L, 0:64],
                                chbf[0:L, va : va + D],
                                start=True,
                                stop=True,
                            )
                            nc.tensor.matmul(
                                av4[64:128, sa : sa + D],
                                pt2[64 : 64 + L, 64:128],
                                chbf[64 : 64 + L, va : va + D],
                                start=True,
                                stop=True,
                            )
                            nc.tensor.matmul(
                                av4[0:64, sb : sb + D],
                                pt2[0:L, 128:192],
                                chbf[0:L, vb : vb + D],
                                start=True,
                                stop=True,
                            )
                            nc.tensor.matmul(
                                av4[64:128, sb : sb + D],
                                pt2[64 : 64 + L, 192:256],
                                chbf[64 : 64 + L, vb : vb + D],
                                start=True,
                                stop=True,
                            )

                            # --- normalize on the forced psum->sbuf copy (Act)
                            nc.scalar.activation(
                                out4[:, sa : sa + D],
                                av4[:, sa : sa + D],
                                mybir.ActivationFunctionType.Copy,
                                bias=0.0,
                                scale=rec2[:, 0:1],
                            )
                            nc.scalar.activation(
                                out4[:, sb : sb + D],
                                av4[:, sb : sb + D],
                                mybir.ActivationFunctionType.Copy,
                                bias=0.0,
                                scale=rec2[:, 1:2],
                            )

                        h0 = c * H_CHUNK + 4 * g
                        dst = out[2 * j : 2 * j + 2, :, h0 : h0 + 4, :]
                        nc.sync.dma_start(
                            out=dst.rearrange("b s h d -> (b s) (h d)"), in_=out4
                        )
    _legalize_waits(nc)
    return nc


def _get_program(L: int) -> bass.Bass:
    if L not in _BUILD_CACHE:
        _BUILD_CACHE[L] = _build(L)
    return _BUILD_CACHE[L]


_RUNNER_CACHE: dict[int, object] = {}


def _make_runner(L: int):
    """Persistent jitted shard_map runner over the 8 cores (mirrors
    concourse.bass2jax.run_bass_via_pjrt, but reusable across calls so
    steady-state executions can be timed without re-tracing)."""
    import jax
    from jax.sharding import Mesh, PartitionSpec
    from jax.experimental.shard_map import shard_map
    from concourse import bass2jax

    bass2jax.install_neuronx_cc_hook()
    nc = _get_program(L)

    out_shape = (B_CORE, SEQ, NUM_HEAD, HEAD_DIM)
    out_aval = jax.core.ShapedArray(out_shape, np.float32)
    part_name = nc.partition_id_tensor.name if nc.partition_id_tensor else None
    in_names = ("qkv", "out") + ((part_name,) if part_name else ())

    def _body(qkv_arr, out_zero):
        operands = [qkv_arr, out_zero]
        if part_name:
            operands.append(bass2jax.partition_id_tensor())
        outs = bass2jax._bass_exec_p.bind(
            *operands,
            out_avals=(out_aval,),
            in_names=in_names,
            out_names=("out",),
            lowering_input_output_aliases=(),
            sim_require_finite=True,
            sim_require_nnan=True,
            nc=nc,
        )
        return outs[0]

    devices = jax.devices()[:N_CORES]
    mesh = Mesh(np.asarray(devices), ("core",))
    sharded = jax.jit(
        shard_map(
            _body,
            mesh=mesh,
            in_specs=(PartitionSpec("core"), PartitionSpec("core")),
            out_specs=PartitionSpec("core"),
            check_rep=False,
        ),
        donate_argnums=(1,),
        keep_unused=True,
    )

    def run(qkv_full: np.ndarray) -> np.ndarray:
        zeros = np.zeros((N_CORES * B_CORE, SEQ, NUM_HEAD, HEAD_DIM), np.float32)
        out = sharded(qkv_full, zeros)
        return np.asarray(out)

    return run


def _get_runner(L: int):
    if L not in _RUNNER_CACHE:
        _RUNNER_CACHE[L] = _make_runner(L)
    return _RUNNER_CACHE[L]


def _run(qkv: np.ndarray, kv_seq_len, trace: bool = False):
    L = int(kv_seq_len)
    L = max(1, min(SEQ, L))
    nc = _get_program(L)
    qkv = np.ascontiguousarray(np.asarray(qkv, dtype=np.float32))
    in_maps = [
        {"qkv": qkv[i * B_CORE : (i + 1) * B_CORE]} for i in range(N_CORES)
    ]
    res = run_bass_kernel_spmd(nc, in_maps, list(range(N_CORES)), trace=trace)
    outs = [res.results[i]["out"] for i in range(N_CORES)]
    full = np.concatenate(outs, axis=0).astype(np.float32)
    return full, res


def kernel(qkv: np.ndarray, kv_seq_len) -> np.ndarray:
    L = max(1, min(SEQ, int(kv_seq_len)))
    qkv = np.ascontiguousarray(np.asarray(qkv, dtype=np.float32))
    return _get_runner(L)(qkv)


# revision 9
# speedup vs baseline: 86.3923x; 86.3923x over previous
"""Trainium2 Bass kernel: batched multi-head attention with padded KV.

Problem shape (hardcoded): qkv [128, 64, 32, 384] f32 packed Q|K|V on the
last axis, head_dim 128, kv_seq_len scalar (<= 64). Output [128, 64, 32, 128].

Sharding: data-parallel over the request (batch) axis across 8 NeuronCores
(16 requests per core). Each core runs the same SPMD program on its slice.

Per-core pipeline, per pair of heads (2 requests stacked on partitions):
  DMA qkv chunk -> cast f32->bf16 (gpsimd) -> PE transpose Q,K (d onto
  partitions) -> psum->sbuf copy (DVE) -> scores matmul (PE) -> exp with
  accumulated denominator (Act) -> reciprocal (DVE) -> PE transpose of the
  exp matrix -> AV matmul (PE) -> normalize-on-copy psum->sbuf (Act) -> DMA.
"""

from contextlib import ExitStack

import numpy as np

import bass_rust
import concourse.bass as bass
import concourse.mybir as mybir
import concourse.tile as tile
from concourse.bass_utils import run_bass_kernel_spmd
from concourse.masks import make_identity

NUM_REQ = 128
SEQ = 64
NUM_HEAD = 32
HEAD_DIM = 128
N_CORES = 8
B_CORE = NUM_REQ // N_CORES  # 16 requests per core
N_BLK = B_CORE // 2          # 8 two-request blocks
H_CHUNK = 8                  # heads per DMA chunk
N_CHUNK = NUM_HEAD // H_CHUNK
SCALE = 1.0 / float(np.sqrt(HEAD_DIM))

DT = mybir.dt
F32 = DT.float32
BF16 = DT.bfloat16

_BUILD_CACHE: dict[int, bass.Bass] = {}


def _legalize_waits(nc: bass.Bass, cap_default: int = 1, cap_ev: int = 2) -> int:
    """Walrus codegen accepts at most 1 sync wait per engine instruction
    (2 on InstEventSemaphore). Tile's scheduler attaches more; spill the
    excess into dedicated InstEventSemaphore instructions placed right
    before the owning instruction on the same engine — the engine stream
    is in-order, so blocking at the preceding instruction is equivalent."""
    ctr = 0
    for func in nc.m.functions:
        for blk in func.blocks:
            out = []
            changed = False
            for inst in blk.instructions:
                si = inst.sync_info
                cap = (
                    cap_ev
                    if isinstance(inst, mybir.InstEventSemaphore)
                    else cap_default
                )
                if si is not None:
                    waits = list(si.on_wait)
                    if len(waits) > cap:
                        extra, keep = waits[:-cap], waits[-cap:]
                        for j in range(0, len(extra), 2):
                            ev = mybir.InstEventSemaphore(
                                name=f"I-evw{ctr}", ins=[], outs=[]
                            )
                            ctr += 1
                            ev.engine = inst.engine
                            ev.sync_info = bass_rust.SyncInfo(
                                on_wait=extra[j : j + 2], on_update=[]
                            )
                            out.append(ev)
                        si.on_wait = keep
                        changed = True
                out.append(inst)
            if changed:
                blk.instructions = out
    return ctr


def _build(L: int) -> bass.Bass:
    """Build the per-core SPMD program for active kv length L (1..64)."""
    nc = bass.Bass()
    qkv = nc.declare_dram_parameter(
        "qkv", [B_CORE, SEQ, NUM_HEAD, 3 * HEAD_DIM], F32, isOutput=False
    )
    out = nc.declare_dram_parameter(
        "out", [B_CORE, SEQ, NUM_HEAD, HEAD_DIM], F32, isOutput=True
    )

    with tile.TileContext(nc) as tc:
        with ExitStack() as ctx:
            singles = ctx.enter_context(tc.tile_pool(name="singles", bufs=1))
            pool_in = ctx.enter_context(tc.tile_pool(name="in", bufs=2))
            pool_bf = ctx.enter_context(tc.tile_pool(name="bf", bufs=2))
            pool_qtkt = ctx.enter_context(tc.tile_pool(name="qtkt", bufs=3))
            pool_p = ctx.enter_context(tc.tile_pool(name="p", bufs=3))
            pool_pt = ctx.enter_context(tc.tile_pool(name="pt", bufs=3))
            pool_sm = ctx.enter_context(tc.tile_pool(name="sm", bufs=4))
            pool_out = ctx.enter_context(tc.tile_pool(name="out", bufs=3))
            ps_qtkt = ctx.enter_context(
                tc.tile_pool(name="ps_qtkt", bufs=2, space="PSUM")
            )
            ps_sc = ctx.enter_context(tc.tile_pool(name="ps_sc", bufs=2, space="PSUM"))
            ps_pt = ctx.enter_context(tc.tile_pool(name="ps_pt", bufs=2, space="PSUM"))
            ps_av = ctx.enter_context(tc.tile_pool(name="ps_av", bufs=2, space="PSUM"))

            ident = singles.tile([128, 128], BF16)
            make_identity(nc, ident)

            D = HEAD_DIM
            for j in range(N_BLK):
                for c in range(N_CHUNK):
                    chunk = pool_in.tile([128, H_CHUNK * 3 * D], F32)
                    src = qkv[2 * j : 2 * j + 2, :, c * H_CHUNK : (c + 1) * H_CHUNK, :]
                    nc.gpsimd.dma_start(
                        out=chunk, in_=src.rearrange("b s h d -> (b s) (h d)")
                    )
                    chbf = pool_bf.tile([128, H_CHUNK * 3 * D], BF16)
                    nc.gpsimd.tensor_copy(chbf[:, :], chunk[:, :])

                    for g in range(H_CHUNK // 4):  # groups of 4 heads -> one out DMA
                        out4 = pool_out.tile([128, 4 * D], F32)
                        av4 = ps_av.tile([128, 4 * D], F32)
                        for pi in range(2):  # pairs of heads within the group
                            ha = 4 * g + 2 * pi  # head index within chunk
                            ca = ha * 3 * D      # column base of head a in chunk
                            cb = (ha + 1) * 3 * D
                            # --- transpose Q,K of both heads: d onto partitions
                            qtkt_ps = ps_qtkt.tile([128, 512], BF16)
                            nc.tensor.transpose(
                                qtkt_ps[:, 0:128], chbf[:, ca : ca + D], ident[:, :]
                            )
                            nc.tensor.transpose(
                                qtkt_ps[:, 128:256],
                                chbf[:, ca + D : ca + 2 * D],
                                ident[:, :],
                            )
                            nc.tensor.transpose(
                                qtkt_ps[:, 256:384], chbf[:, cb : cb + D], ident[:, :]
                            )
                            nc.tensor.transpose(
                                qtkt_ps[:, 384:512],
                                chbf[:, cb + D : cb + 2 * D],
                                ident[:, :],
                            )
                            qtkt = pool_qtkt.tile([128, 512], BF16)
                            nc.vector.tensor_copy(qtkt[:, :], qtkt_ps[:, :])

                            # --- scores: [q, k] per (head, request)
                            # layout in sc2: partitions q-cat (b0|b1), free 0:64
                            # head a, 64:128 head b
                            sc2 = ps_sc.tile([128, 128], F32)
                            nc.tensor.matmul(
                                sc2[0:64, 0:L],
                                qtkt[:, 0:64],
                                qtkt[:, 128 : 128 + L],
                                start=True,
                                stop=True,
                            )
                            nc.tensor.matmul(
                                sc2[64:128, 0:L],
                                qtkt[:, 64:128],
                                qtkt[:, 192 : 192 + L],
                                start=True,
                                stop=True,
                            )
                            nc.tensor.matmul(
                                sc2[0:64, 64 : 64 + L],
                                qtkt[:, 256:320],
                                qtkt[:, 384 : 384 + L],
                                start=True,
                                stop=True,
                            )
                            nc.tensor.matmul(
                                sc2[64:128, 64 : 64 + L],
                                qtkt[:, 320:384],
                                qtkt[:, 448 : 448 + L],
                                start=True,
                                stop=True,
                            )

                            # --- exp(scale * s), accumulating the denominator
                            p2 = pool_p.tile([128, 128], BF16)
                            den2 = pool_sm.tile([128, 2], F32)
                            nc.scalar.activation(
                                p2[:, 0:L],
                                sc2[:, 0:L],
                                mybir.ActivationFunctionType.Exp,
                                bias=0.0,
                                scale=SCALE,
                                accum_out=den2[:, 0:1],
                            )
                            nc.scalar.activation(
                                p2[:, 64 : 64 + L],
                                sc2[:, 64 : 64 + L],
                                mybir.ActivationFunctionType.Exp,
                                bias=0.0,
                                scale=SCALE,
                                accum_out=den2[:, 1:2],
                            )
                            rec2 = pool_sm.tile([128, 2], F32)
                            nc.vector.reciprocal(rec2[:, :], den2[:, :])

                            # --- transpose p: k onto partitions
                            pt_ps = ps_pt.tile([64, 256], BF16)
                            nc.tensor.transpose(
                                pt_ps[0:L, 0:128], p2[:, 0:L], ident[:, :]
                            )
                            nc.tensor.transpose(
                                pt_ps[0:L, 128:256], p2[:, 64 : 64 + L], ident[:, :]
                            )
                            pt2 = pool_pt.tile([128, 256], BF16)
                            src_b0 = pt_ps[0:L].rearrange("p (t q) -> p t q", t=2)
                            dst_b0 = pt2[0:L].rearrange("p (t q) -> p t q", t=2)
                            nc.vector.tensor_copy(
                                dst_b0[:, :, 0:64], src_b0[:, :, 0:64]
                            )
                            src_b1 = pt_ps[0:L].rearrange("p (t q) -> p t q", t=2)
                            dst_b1 = pt2[64 : 64 + L].rearrange(
                                "p (t q) -> p t q", t=2
                            )
                            nc.vector.tensor_copy(
                                dst_b1[:, :, 64:128], src_b1[:, :, 64:128]
                            )

                            # --- attn @ V, unnormalized, into the group psum
                            sa = 2 * pi * D
                            sb = sa + D
                            va = ca + 2 * D
                            vb = cb + 2 * D
                            nc.tensor.matmul(
                                av4[0:64, sa : sa + D],
                                pt2[0:L, 0:64],
                                chbf[0:L, va : va + D],
                                start=True,
                                stop=True,
                            )
                            nc.tensor.matmul(
                                av4[64:128, sa : sa + D],
                                pt2[64 : 64 + L, 64:128],
                                chbf[64 : 64 + L, va : va + D],
                                start=True,
                                stop=True,
                            )
                            nc.tensor.matmul(
                                av4[0:64, sb : sb + D],
                                pt2[0:L, 128:192],
                                chbf[0:L, vb : vb + D],
                                start=True,
                                stop=True,
                            )
                            nc.tensor.matmul(
                                av4[64:128, sb : sb + D],
                                pt2[64 : 64 + L, 192:256],
                                chbf[64 : 64 + L, vb : vb + D],
                                start=True,
                                stop=True,
                            )

                            # --- normalize on the forced psum->sbuf copy (Act)
                            nc.scalar.activation(
                                out4[:, sa : sa + D],
                                av4[:, sa : sa + D],
                                mybir.ActivationFunctionType.Copy,
                                bias=0.0,
                                scale=rec2[:, 0:1],
                            )
                            nc.scalar.activation(
                                out4[:, sb : sb + D],
                                av4[:, sb : sb + D],
                                mybir.ActivationFunctionType.Copy,
                                bias=0.0,
                                scale=rec2[:, 1:2],
                            )

                        h0 = c * H_CHUNK + 4 * g
                        dst = out[2 * j : 2 * j + 2, :, h0 : h0 + 4, :]
                        nc.sync.dma_start(
                            out=dst.rearrange("b s h d -> (b s) (h d)"), in_=out4
                        )
    _legalize_waits(nc)
    return nc


def _get_program(L: int) -> bass.Bass:
    if L not in _BUILD_CACHE:
        _BUILD_CACHE[L] = _build(L)
    return _BUILD_CACHE[L]


_RUNNER_CACHE: dict[int, object] = {}


def _make_runner(L: int):
    """Persistent jitted shard_map runner over the 8 cores (mirrors
    concourse.bass2jax.run_bass_via_pjrt, but reusable across calls so
    steady-state executions can be timed without re-tracing)."""
    import jax
    from jax.sharding import Mesh, PartitionSpec
    from jax.experimental.shard_map import shard_map
    from concourse import bass2jax

    bass2jax.install_neuronx_cc_hook()
    nc = _get_program(L)

    out_shape = (B_CORE, SEQ, NUM_HEAD, HEAD_DIM)
    out_aval = jax.core.ShapedArray(out_shape, np.float32)
    part_name = nc.partition_id_tensor.name if nc.partition_id_tensor else None
    in_names = ("qkv", "out") + ((part_name,) if part_name else ())

    def _body(qkv_arr, out_zero):
        operands = [qkv_arr, out_zero]
        if part_name:
            operands.append(bass2jax.partition_id_tensor())
        outs = bass2jax._bass_exec_p.bind(
            *operands,
            out_avals=(out_aval,),
            in_names=in_names,
            out_names=("out",),
            lowering_input_output_aliases=(),
            sim_require_finite=True,
            sim_require_nnan=True,
            nc=nc,
        )
        return outs[0]

    devices = jax.devices()[:N_CORES]
    mesh = Mesh(np.asarray(devices), ("core",))
    sharded = jax.jit(
        shard_map(
            _body,
            mesh=mesh,
            in_specs=(PartitionSpec("core"), PartitionSpec("core")),
            out_specs=PartitionSpec("core"),
            check_rep=False,
        ),
        donate_argnums=(1,),
        keep_unused=True,
    )

    def run(qkv_full: np.ndarray) -> np.ndarray:
        zeros = np.zeros((N_CORES * B_CORE, SEQ, NUM_HEAD, HEAD_DIM), np.float32)
        out = sharded(qkv_full, zeros)
        return np.asarray(out)

    run.sharded = sharded
    run.mesh = mesh
    run.out_shape = (N_CORES * B_CORE, SEQ, NUM_HEAD, HEAD_DIM)
    return run


def _get_runner(L: int):
    if L not in _RUNNER_CACHE:
        _RUNNER_CACHE[L] = _make_runner(L)
    return _RUNNER_CACHE[L]


def _run(qkv: np.ndarray, kv_seq_len, trace: bool = False):
    L = int(kv_seq_len)
    L = max(1, min(SEQ, L))
    nc = _get_program(L)
    qkv = np.ascontiguousarray(np.asarray(qkv, dtype=np.float32))
    in_maps = [
        {"qkv": qkv[i * B_CORE : (i + 1) * B_CORE]} for i in range(N_CORES)
    ]
    res = run_bass_kernel_spmd(nc, in_maps, list(range(N_CORES)), trace=trace)
    outs = [res.results[i]["out"] for i in range(N_CORES)]
    full = np.concatenate(outs, axis=0).astype(np.float32)
    return full, res


def kernel(qkv: np.ndarray, kv_seq_len) -> np.ndarray:
    L = max(1, min(SEQ, int(kv_seq_len)))
    qkv = np.ascontiguousarray(np.asarray(qkv, dtype=np.float32))
    return _get_runner(L)(qkv)


# revision 19
# speedup vs baseline: 823.9080x; 9.5368x over previous
"""Trainium2 Bass kernel: batched multi-head attention with padded KV.

Problem shape (hardcoded): qkv [128, 64, 32, 384] f32 packed Q|K|V on the
last axis, head_dim 128, kv_seq_len scalar (<= 64). Output [128, 64, 32, 128].

Sharding: data-parallel over the request (batch) axis across 8 NeuronCores
(16 requests per core). Each core runs the same SPMD program on its slice.

Per-core pipeline, per pair of heads (2 requests stacked on partitions):
  DMA qkv chunk -> cast f32->bf16 (gpsimd) -> PE transpose Q,K (d onto
  partitions) -> psum->sbuf copy (DVE) -> scores matmul (PE) -> exp with
  accumulated denominator (Act) -> reciprocal (DVE) -> PE transpose of the
  exp matrix -> AV matmul (PE) -> normalize-on-copy psum->sbuf (Act) -> DMA.
"""

from contextlib import ExitStack

import numpy as np

import bass_rust
import concourse.bass as bass
import concourse.mybir as mybir
import concourse.tile as tile
from concourse.bass_utils import run_bass_kernel_spmd
from concourse.masks import make_identity

NUM_REQ = 128
SEQ = 64
NUM_HEAD = 32
HEAD_DIM = 128
N_CORES = 8
B_CORE = NUM_REQ // N_CORES  # 16 requests per core
N_BLK = B_CORE // 2          # 8 two-request blocks
H_CHUNK = 8                  # heads per DMA chunk
N_CHUNK = NUM_HEAD // H_CHUNK
SCALE = 1.0 / float(np.sqrt(HEAD_DIM))

DT = mybir.dt
F32 = DT.float32
BF16 = DT.bfloat16

_BUILD_CACHE: dict[int, bass.Bass] = {}


def _legalize_waits(nc: bass.Bass, cap_default: int = 1, cap_ev: int = 2) -> int:
    """Walrus codegen accepts at most 1 sync wait per engine instruction
    (2 on InstEventSemaphore). Tile's scheduler attaches more; spill the
    excess into dedicated InstEventSemaphore instructions placed right
    before the owning instruction on the same engine — the engine stream
    is in-order, so blocking at the preceding instruction is equivalent."""
    ctr = 0
    for func in nc.m.functions:
        for blk in func.blocks:
            out = []
            changed = False
            for inst in blk.instructions:
                si = inst.sync_info
                cap = (
                    cap_ev
                    if isinstance(inst, mybir.InstEventSemaphore)
                    else cap_default
                )
                if si is not None:
                    waits = list(si.on_wait)
                    if len(waits) > cap:
                        extra, keep = waits[:-cap], waits[-cap:]
                        for j in range(0, len(extra), 2):
                            ev = mybir.InstEventSemaphore(
                                name=f"I-evw{ctr}", ins=[], outs=[]
                            )
                            ctr += 1
                            ev.engine = inst.engine
                            ev.sync_info = bass_rust.SyncInfo(
                                on_wait=extra[j : j + 2], on_update=[]
                            )
                            out.append(ev)
                        si.on_wait = keep
                        changed = True
                out.append(inst)
            if changed:
                blk.instructions = out
    return ctr


def _build(L: int, repeat: int = 1) -> bass.Bass:
    """Build the per-core SPMD program for active kv length L (1..64).

    repeat > 1 re-runs the whole computation that many times (identical
    output) — used only for slope-based device timing."""
    nc = bass.Bass()
    qkv = nc.declare_dram_parameter(
        "qkv", [B_CORE, SEQ, NUM_HEAD, 3 * HEAD_DIM], F32, isOutput=False
    )
    out = nc.declare_dram_parameter(
        "out", [B_CORE, SEQ, NUM_HEAD, HEAD_DIM], F32, isOutput=True
    )

    with tile.TileContext(nc) as tc:
        with ExitStack() as ctx:
            singles = ctx.enter_context(tc.tile_pool(name="singles", bufs=1))
            pool_in = ctx.enter_context(tc.tile_pool(name="in", bufs=3))
            pool_qk = ctx.enter_context(tc.tile_pool(name="qk", bufs=2))
            pool_v = ctx.enter_context(tc.tile_pool(name="v", bufs=2))
            pool_qtkt = ctx.enter_context(tc.tile_pool(name="qtkt", bufs=3))
            pool_p = ctx.enter_context(tc.tile_pool(name="p", bufs=3))
            pool_pt = ctx.enter_context(tc.tile_pool(name="pt", bufs=3))
            pool_sm = ctx.enter_context(tc.tile_pool(name="sm", bufs=4))
            pool_out = ctx.enter_context(tc.tile_pool(name="out", bufs=3))
            ps_qtkt = ctx.enter_context(
                tc.tile_pool(name="ps_qtkt", bufs=2, space="PSUM")
            )
            ps_sc = ctx.enter_context(tc.tile_pool(name="ps_sc", bufs=2, space="PSUM"))
            ps_pt = ctx.enter_context(tc.tile_pool(name="ps_pt", bufs=2, space="PSUM"))
            ps_av = ctx.enter_context(tc.tile_pool(name="ps_av", bufs=2, space="PSUM"))

            ident = singles.tile([128, 128], BF16)
            make_identity(nc, ident)

            D = HEAD_DIM

            def _emit_body():
              for j in range(N_BLK):
                for c in range(N_CHUNK):
                    chunk = pool_in.tile([128, H_CHUNK * 3 * D], F32)
                    src = qkv[2 * j : 2 * j + 2, :, c * H_CHUNK : (c + 1) * H_CHUNK, :]
                    nc.sync.dma_start(
                        out=chunk, in_=src.rearrange("b s h d -> (b s) (h d)")
                    )
                    ch3 = chunk[:].rearrange("p (h x) -> p h x", h=H_CHUNK)
                    # Q,K cast on the Pool engine (compact [h, 256] layout)
                    chqk = pool_qk.tile([128, H_CHUNK, 2 * D], BF16)
                    nc.gpsimd.tensor_copy(chqk[:, :, :], ch3[:, :, 0 : 2 * D])
                    # V cast on DVE into [h, 129] tiles; ones column for the
                    # softmax denominators via the AV matmul
                    chv = pool_v.tile([128, H_CHUNK, D + 1], BF16)
                    nc.vector.tensor_copy(chv[:, :, 0:D], ch3[:, :, 2 * D : 3 * D])
                    nc.gpsimd.memset(chv[:, :, D : D + 1], 1.0)

                    for g in range(H_CHUNK // 4):  # groups of 4 heads
                        out4 = pool_out.tile([128, 4 * D], F32)
                        # Q,K transposes for 4 heads -> one psum bank and one
                        # psum->sbuf copy
                        qtkt_ps = ps_qtkt.tile([128, 8 * D], BF16)
                        for hh in range(4):
                            h = 4 * g + hh
                            nc.tensor.transpose(
                                qtkt_ps[:, 2 * hh * D : (2 * hh + 1) * D],
                                chqk[:, h, 0:D],
                                ident[:, :],
                            )
                            nc.tensor.transpose(
                                qtkt_ps[:, (2 * hh + 1) * D : (2 * hh + 2) * D],
                                chqk[:, h, D : 2 * D],
                                ident[:, :],
                            )
                        qtkt = pool_qtkt.tile([128, 8 * D], BF16)
                        nc.vector.tensor_copy(qtkt[:, :], qtkt_ps[:, :])

                        pt_ps = ps_pt.tile([64, 4 * D], BF16)
                        avs = []
                        for pi in range(2):  # pairs of heads within the group
                            qa = 2 * pi * 2 * D
                            qb = (2 * pi + 1) * 2 * D
                            # --- scores [q-cat(b0|b1), k]; head a cols 0:64,
                            # head b cols 64:128
                            sc2 = ps_sc.tile([128, 128], F32)
                            nc.tensor.matmul(
                                sc2[0:64, 0:L],
                                qtkt[:, qa : qa + 64],
                                qtkt[:, qa + D : qa + D + L],
                                start=True,
                                stop=True,
                            )
                            nc.tensor.matmul(
                                sc2[64:128, 0:L],
                                qtkt[:, qa + 64 : qa + D],
                                qtkt[:, qa + D + 64 : qa + D + 64 + L],
                                start=True,
                                stop=True,
                            )
                            nc.tensor.matmul(
                                sc2[0:64, 64 : 64 + L],
                                qtkt[:, qb : qb + 64],
                                qtkt[:, qb + D : qb + D + L],
                                start=True,
                                stop=True,
                            )
                            nc.tensor.matmul(
                                sc2[64:128, 64 : 64 + L],
                                qtkt[:, qb + 64 : qb + D],
                                qtkt[:, qb + D + 64 : qb + D + 64 + L],
                                start=True,
                                stop=True,
                            )

                            # --- one exp for both heads (denominators come
                            # from the ones column in the AV matmul)
                            p2 = pool_p.tile([128, 128], BF16)
                            sc3 = sc2[:].rearrange("p (t k) -> p t k", t=2)
                            p3 = p2[:].rearrange("p (t k) -> p t k", t=2)
                            nc.scalar.activation(
                                p3[:, :, 0:L],
                                sc3[:, :, 0:L],
                                mybir.ActivationFunctionType.Exp,
                                bias=0.0,
                                scale=SCALE,
                            )

                            # --- transpose p: k onto partitions
                            nc.tensor.transpose(
                                pt_ps[0:L, 2 * pi * D : 2 * pi * D + D],
                                p2[:, 0:L],
                                ident[:, :],
                            )
                            nc.tensor.transpose(
                                pt_ps[0:L, (2 * pi + 1) * D : (2 * pi + 2) * D],
                                p2[:, 64 : 64 + L],
                                ident[:, :],
                            )

                        # --- p^T psum->sbuf for the whole group: one strided
                        # copy for the b0 blocks, one for b1
                        pt4 = pool_pt.tile([128, 4 * D], BF16)
                        src4 = pt_ps[0:L].rearrange("p (t q) -> p t q", t=4)
                        dst0 = pt4[0:L].rearrange("p (t q) -> p t q", t=4)
                        dst1 = pt4[64 : 64 + L].rearrange("p (t q) -> p t q", t=4)
                        nc.vector.tensor_copy(dst0[:, :, 0:64], src4[:, :, 0:64])
                        nc.vector.tensor_copy(dst1[:, :, 64:128], src4[:, :, 64:128])

                        for pi in range(2):  # attn @ [V|1] per pair
                            av2 = ps_av.tile([128, 2, D + 1], F32)
                            for i in range(2):
                                hh = 2 * pi + i
                                h = 4 * g + hh
                                so = hh * D
                                nc.tensor.matmul(
                                    av2[0:64, i, :],
                                    pt4[0:L, so : so + 64],
                                    chv[0:L, h, :],
                                    start=True,
                                    stop=True,
                                )
                                nc.tensor.matmul(
                                    av2[64:128, i, :],
                                    pt4[64 : 64 + L, so + 64 : so + D],
                                    chv[64 : 64 + L, h, :],
                                    start=True,
                                    stop=True,
                                )
                            rec2 = pool_sm.tile([128, 2], F32)
                            nc.vector.reciprocal(rec2[:, :], av2[:, :, D])
                            for i in range(2):
                                hh = 2 * pi + i
                                so = hh * D
                                nc.scalar.activation(
                                    out4[:, so : so + D],
                                    av2[:, i, 0:D],
                                    mybir.ActivationFunctionType.Copy,
                                    bias=0.0,
                                    scale=rec2[:, i : i + 1],
                                )

                        h0 = c * H_CHUNK + 4 * g
                        dst = out[2 * j : 2 * j + 2, :, h0 : h0 + 4, :]
                        nc.sync.dma_start(
                            out=dst.rearrange("b s h d -> (b s) (h d)"), in_=out4
                        )

            if repeat == 1:
                _emit_body()
            else:
                with tc.For_i(0, repeat, 1):
                    _emit_body()
    _legalize_waits(nc)
    return nc


def _get_program(L: int, repeat: int = 1) -> bass.Bass:
    key = (L, repeat)
    if key not in _BUILD_CACHE:
        _BUILD_CACHE[key] = _build(L, repeat)
    return _BUILD_CACHE[key]


_RUNNER_CACHE: dict[int, object] = {}


def _make_runner(L: int, repeat: int = 1):
    """Persistent jitted shard_map runner over the 8 cores (mirrors
    concourse.bass2jax.run_bass_via_pjrt, but reusable across calls so
    steady-state executions can be timed without re-tracing)."""
    import jax
    from jax.sharding import Mesh, PartitionSpec
    from jax.experimental.shard_map import shard_map
    from concourse import bass2jax

    bass2jax.install_neuronx_cc_hook()
    nc = _get_program(L, repeat)

    out_shape = (B_CORE, SEQ, NUM_HEAD, HEAD_DIM)
    out_aval = jax.core.ShapedArray(out_shape, np.float32)
    part_name = nc.partition_id_tensor.name if nc.partition_id_tensor else None
    in_names = ("qkv", "out") + ((part_name,) if part_name else ())

    def _body(qkv_arr, out_zero):
        operands = [qkv_arr, out_zero]
        if part_name:
            operands.append(bass2jax.partition_id_tensor())
        outs = bass2jax._bass_exec_p.bind(
            *operands,
            out_avals=(out_aval,),
            in_names=in_names,
            out_names=("out",),
            lowering_input_output_aliases=(),
            sim_require_finite=True,
            sim_require_nnan=True,
            nc=nc,
        )
        return outs[0]

    devices = jax.devices()[:N_CORES]
    mesh = Mesh(np.asarray(devices), ("core",))
    sharded = jax.jit(
        shard_map(
            _body,
            mesh=mesh,
            in_specs=(PartitionSpec("core"), PartitionSpec("core")),
            out_specs=PartitionSpec("core"),
            check_rep=False,
        ),
        donate_argnums=(1,),
        keep_unused=True,
    )

    def run(qkv_full: np.ndarray) -> np.ndarray:
        zeros = np.zeros((N_CORES * B_CORE, SEQ, NUM_HEAD, HEAD_DIM), np.float32)
        out = sharded(qkv_full, zeros)
        return np.asarray(out)

    run.sharded = sharded
    run.mesh = mesh
    run.out_shape = (N_CORES * B_CORE, SEQ, NUM_HEAD, HEAD_DIM)
    return run


def _get_runner(L: int, repeat: int = 1):
    key = (L, repeat)
    if key not in _RUNNER_CACHE:
        _RUNNER_CACHE[key] = _make_runner(L, repeat)
    return _RUNNER_CACHE[key]


def _run(qkv: np.ndarray, kv_seq_len, trace: bool = False):
    L = int(kv_seq_len)
    L = max(1, min(SEQ, L))
    nc = _get_program(L)
    qkv = np.ascontiguousarray(np.asarray(qkv, dtype=np.float32))
    in_maps = [
        {"qkv": qkv[i * B_CORE : (i + 1) * B_CORE]} for i in range(N_CORES)
    ]
    res = run_bass_kernel_spmd(nc, in_maps, list(range(N_CORES)), trace=trace)
    outs = [res.results[i]["out"] for i in range(N_CORES)]
    full = np.concatenate(outs, axis=0).astype(np.float32)
    return full, res


def kernel(qkv: np.ndarray, kv_seq_len) -> np.ndarray:
    L = max(1, min(SEQ, int(kv_seq_len)))
    qkv = np.ascontiguousarray(np.asarray(qkv, dtype=np.float32))
    return _get_runner(L)(qkv)


# revision 23
# speedup vs baseline: 40567.2086x; 49.2375x over previous
"""Trainium2 Bass kernel: batched multi-head attention with padded KV.

Problem shape (hardcoded): qkv [128, 64, 32, 384] f32 packed Q|K|V on the
last axis, head_dim 128, kv_seq_len scalar (<= 64). Output [128, 64, 32, 128].

Sharding: data-parallel over the request (batch) axis across 8 NeuronCores
(16 requests per core). Each core runs the same SPMD program on its slice.

Per-core pipeline, per pair of heads (2 requests stacked on partitions):
  DMA qkv chunk -> cast f32->bf16 (gpsimd) -> PE transpose Q,K (d onto
  partitions) -> psum->sbuf copy (DVE) -> scores matmul (PE) -> exp with
  accumulated denominator (Act) -> reciprocal (DVE) -> PE transpose of the
  exp matrix -> AV matmul (PE) -> normalize-on-copy psum->sbuf (Act) -> DMA.
"""

from contextlib import ExitStack

import numpy as np

import bass_rust
import concourse.bass as bass
import concourse.mybir as mybir
import concourse.tile as tile
from concourse.bass_utils import run_bass_kernel_spmd
from concourse.masks import make_identity

NUM_REQ = 128
SEQ = 64
NUM_HEAD = 32
HEAD_DIM = 128
N_CORES = 8
B_CORE = NUM_REQ // N_CORES  # 16 requests per core
N_BLK = B_CORE // 2          # 8 two-request blocks
H_CHUNK = 8                  # heads per DMA chunk
N_CHUNK = NUM_HEAD // H_CHUNK
SCALE = 1.0 / float(np.sqrt(HEAD_DIM))

DT = mybir.dt
F32 = DT.float32
BF16 = DT.bfloat16

_BUILD_CACHE: dict[int, bass.Bass] = {}


def _legalize_waits(nc: bass.Bass, cap_default: int = 1, cap_ev: int = 2) -> int:
    """Walrus codegen accepts at most 1 sync wait per engine instruction
    (2 on InstEventSemaphore). Tile's scheduler attaches more; spill the
    excess into dedicated InstEventSemaphore instructions placed right
    before the owning instruction on the same engine — the engine stream
    is in-order, so blocking at the preceding instruction is equivalent."""
    ctr = 0
    for func in nc.m.functions:
        for blk in func.blocks:
            out = []
            changed = False
            for inst in blk.instructions:
                si = inst.sync_info
                cap = (
                    cap_ev
                    if isinstance(inst, mybir.InstEventSemaphore)
                    else cap_default
                )
                if si is not None:
                    waits = list(si.on_wait)
                    if len(waits) > cap:
                        extra, keep = waits[:-cap], waits[-cap:]
                        for j in range(0, len(extra), 2):
                            ev = mybir.InstEventSemaphore(
                                name=f"I-evw{ctr}", ins=[], outs=[]
                            )
                            ctr += 1
                            ev.engine = inst.engine
                            ev.sync_info = bass_rust.SyncInfo(
                                on_wait=extra[j : j + 2], on_update=[]
                            )
                            out.append(ev)
                        si.on_wait = keep
                        changed = True
                out.append(inst)
            if changed:
                blk.instructions = out
    return ctr


def _build(L: int, repeat: int = 1, cfg: dict | None = None) -> bass.Bass:
    """Build the per-core SPMD program for active kv length L (1..64).

    repeat > 1 re-runs the whole computation that many times (identical
    output) — used only for slope-based device timing."""
    cfg = cfg or {}
    nc = bass.Bass()
    qkv = nc.declare_dram_parameter(
        "qkv", [B_CORE, SEQ, NUM_HEAD, 3 * HEAD_DIM], F32, isOutput=False
    )
    out = nc.declare_dram_parameter(
        "out", [B_CORE, SEQ, NUM_HEAD, HEAD_DIM], F32, isOutput=True
    )

    with tile.TileContext(nc) as tc:
        with ExitStack() as ctx:
            singles = ctx.enter_context(tc.tile_pool(name="singles", bufs=1))
            pool_in = ctx.enter_context(tc.tile_pool(name="in", bufs=cfg.get("in", 3)))
            pool_qk = ctx.enter_context(tc.tile_pool(name="qk", bufs=cfg.get("qk", 3)))
            pool_v = ctx.enter_context(tc.tile_pool(name="v", bufs=cfg.get("v", 3)))
            pool_qtkt = ctx.enter_context(tc.tile_pool(name="qtkt", bufs=cfg.get("qtkt", 3)))
            pool_p = ctx.enter_context(tc.tile_pool(name="p", bufs=cfg.get("p", 3)))
            pool_pt = ctx.enter_context(tc.tile_pool(name="pt", bufs=cfg.get("pt", 3)))
            pool_sm = ctx.enter_context(tc.tile_pool(name="sm", bufs=cfg.get("sm", 4)))
            pool_out = ctx.enter_context(tc.tile_pool(name="out", bufs=cfg.get("out", 3)))
            ps_qtkt = ctx.enter_context(
                tc.tile_pool(name="ps_qtkt", bufs=cfg.get("ps_qtkt", 2), space="PSUM")
            )
            ps_sc = ctx.enter_context(tc.tile_pool(name="ps_sc", bufs=cfg.get("ps_sc", 2), space="PSUM"))
            ps_pt = ctx.enter_context(tc.tile_pool(name="ps_pt", bufs=cfg.get("ps_pt", 2), space="PSUM"))
            ps_av = ctx.enter_context(tc.tile_pool(name="ps_av", bufs=cfg.get("ps_av", 2), space="PSUM"))

            ident = singles.tile([128, 128], BF16)
            make_identity(nc, ident)

            D = HEAD_DIM

            def _emit_body():
              for j in range(N_BLK):
                for c in range(N_CHUNK):
                    chunk = pool_in.tile([128, H_CHUNK * 3 * D], F32)
                    src = qkv[2 * j : 2 * j + 2, :, c * H_CHUNK : (c + 1) * H_CHUNK, :]
                    nc.sync.dma_start(
                        out=chunk, in_=src.rearrange("b s h d -> (b s) (h d)")
                    )
                    ch3 = chunk[:].rearrange("p (h x) -> p h x", h=H_CHUNK)
                    # Q,K cast on the Pool engine (compact [h, 256] layout)
                    chqk = pool_qk.tile([128, H_CHUNK, 2 * D], BF16)
                    nc.gpsimd.tensor_copy(chqk[:, :, :], ch3[:, :, 0 : 2 * D])
                    # V cast on DVE into [h, 129] tiles; ones column for the
                    # softmax denominators via the AV matmul
                    chv = pool_v.tile([128, H_CHUNK, D + 1], BF16)
                    nc.vector.tensor_copy(chv[:, :, 0:D], ch3[:, :, 2 * D : 3 * D])
                    nc.gpsimd.memset(chv[:, :, D : D + 1], 1.0)

                    for g in range(H_CHUNK // 4):  # groups of 4 heads
                        out4 = pool_out.tile([128, 4 * D], F32)
                        # Q,K transposes for 4 heads -> one psum bank and one
                        # psum->sbuf copy
                        qtkt_ps = ps_qtkt.tile([128, 8 * D], BF16)
                        for hh in range(4):
                            h = 4 * g + hh
                            nc.tensor.transpose(
                                qtkt_ps[:, 2 * hh * D : (2 * hh + 1) * D],
                                chqk[:, h, 0:D],
                                ident[:, :],
                            )
                            nc.tensor.transpose(
                                qtkt_ps[:, (2 * hh + 1) * D : (2 * hh + 2) * D],
                                chqk[:, h, D : 2 * D],
                                ident[:, :],
                            )
                        qtkt = pool_qtkt.tile([128, 8 * D], BF16)
                        nc.vector.tensor_copy(qtkt[:, :], qtkt_ps[:, :])

                        pt_ps = ps_pt.tile([64, 4 * D], BF16)
                        for pi in range(2):  # pairs of heads within the group
                            qa = 2 * pi * 2 * D
                            qb = (2 * pi + 1) * 2 * D
                            # --- scores [q-cat(b0|b1), k]; head a cols 0:64,
                            # head b cols 64:128
                            sc2 = ps_sc.tile([128, 128], F32)
                            nc.tensor.matmul(
                                sc2[0:64, 0:L],
                                qtkt[:, qa : qa + 64],
                                qtkt[:, qa + D : qa + D + L],
                                start=True,
                                stop=True,
                            )
                            nc.tensor.matmul(
                                sc2[64:128, 0:L],
                                qtkt[:, qa + 64 : qa + D],
                                qtkt[:, qa + D + 64 : qa + D + 64 + L],
                                start=True,
                                stop=True,
                            )
                            nc.tensor.matmul(
                                sc2[0:64, 64 : 64 + L],
                                qtkt[:, qb : qb + 64],
                                qtkt[:, qb + D : qb + D + L],
                                start=True,
                                stop=True,
                            )
                            nc.tensor.matmul(
                                sc2[64:128, 64 : 64 + L],
                                qtkt[:, qb + 64 : qb + D],
                                qtkt[:, qb + D + 64 : qb + D + 64 + L],
                                start=True,
                                stop=True,
                            )

                            # --- one exp for both heads (denominators come
                            # from the ones column in the AV matmul)
                            p2 = pool_p.tile([128, 128], BF16)
                            sc3 = sc2[:].rearrange("p (t k) -> p t k", t=2)
                            p3 = p2[:].rearrange("p (t k) -> p t k", t=2)
                            nc.scalar.activation(
                                p3[:, :, 0:L],
                                sc3[:, :, 0:L],
                                mybir.ActivationFunctionType.Exp,
                                bias=0.0,
                                scale=SCALE,
                            )

                            # --- transpose p: k onto partitions
                            nc.tensor.transpose(
                                pt_ps[0:L, 2 * pi * D : 2 * pi * D + D],
                                p2[:, 0:L],
                                ident[:, :],
                            )
                            nc.tensor.transpose(
                                pt_ps[0:L, (2 * pi + 1) * D : (2 * pi + 2) * D],
                                p2[:, 64 : 64 + L],
                                ident[:, :],
                            )

                        # --- p^T psum->sbuf for the whole group: one strided
                        # copy for the b0 blocks, one for b1
                        pt4 = pool_pt.tile([128, 4 * D], BF16)
                        src4 = pt_ps[0:L].rearrange("p (t q) -> p t q", t=4)
                        dst0 = pt4[0:L].rearrange("p (t q) -> p t q", t=4)
                        dst1 = pt4[64 : 64 + L].rearrange("p (t q) -> p t q", t=4)
                        nc.vector.tensor_copy(dst0[:, :, 0:64], src4[:, :, 0:64])
                        nc.vector.tensor_copy(dst1[:, :, 64:128], src4[:, :, 64:128])

                        for pi in range(2):  # attn @ [V|1] per pair
                            av2 = ps_av.tile([128, 2, D + 1], F32)
                            for i in range(2):
                                hh = 2 * pi + i
                                h = 4 * g + hh
                                so = hh * D
                                nc.tensor.matmul(
                                    av2[0:64, i, :],
                                    pt4[0:L, so : so + 64],
                                    chv[0:L, h, :],
                                    start=True,
                                    stop=True,
                                )
                                nc.tensor.matmul(
                                    av2[64:128, i, :],
                                    pt4[64 : 64 + L, so + 64 : so + D],
                                    chv[64 : 64 + L, h, :],
                                    start=True,
                                    stop=True,
                                )
                            rec2 = pool_sm.tile([128, 2], F32)
                            nc.vector.reciprocal(rec2[:, :], av2[:, :, D])
                            for i in range(2):
                                hh = 2 * pi + i
                                so = hh * D
                                nc.scalar.activation(
                                    out4[:, so : so + D],
                                    av2[:, i, 0:D],
                                    mybir.ActivationFunctionType.Copy,
                                    bias=0.0,
                                    scale=rec2[:, i : i + 1],
                                )

                        h0 = c * H_CHUNK + 4 * g
                        dst = out[2 * j : 2 * j + 2, :, h0 : h0 + 4, :]
                        out_eng = {
                            "sp": nc.sync,
                            "act": nc.scalar,
                            "pool": nc.gpsimd,
                        }[cfg.get("out_dma", "sp")]
                        out_eng.dma_start(
                            out=dst.rearrange("b s h d -> (b s) (h d)"), in_=out4
                        )

            if repeat == 1:
                _emit_body()
            else:
                with tc.For_i(0, repeat, 1):
                    _emit_body()
    _legalize_waits(nc)
    return nc


def _get_program(L: int, repeat: int = 1) -> bass.Bass:
    key = (L, repeat)
    if key not in _BUILD_CACHE:
        _BUILD_CACHE[key] = _build(L, repeat)
    return _BUILD_CACHE[key]


_RUNNER_CACHE: dict[int, object] = {}


def _make_runner(L: int, repeat: int = 1):
    """Persistent jitted shard_map runner over the 8 cores (mirrors
    concourse.bass2jax.run_bass_via_pjrt, but reusable across calls so
    steady-state executions can be timed without re-tracing)."""
    import jax
    from jax.sharding import Mesh, PartitionSpec
    from jax.experimental.shard_map import shard_map
    from concourse import bass2jax

    bass2jax.install_neuronx_cc_hook()
    nc = _get_program(L, repeat)

    out_shape = (B_CORE, SEQ, NUM_HEAD, HEAD_DIM)
    out_aval = jax.core.ShapedArray(out_shape, np.float32)
    part_name = nc.partition_id_tensor.name if nc.partition_id_tensor else None
    in_names = ("qkv", "out") + ((part_name,) if part_name else ())

    def _body(qkv_arr, out_zero):
        operands = [qkv_arr, out_zero]
        if part_name:
            operands.append(bass2jax.partition_id_tensor())
        outs = bass2jax._bass_exec_p.bind(
            *operands,
            out_avals=(out_aval,),
            in_names=in_names,
            out_names=("out",),
            lowering_input_output_aliases=(),
            sim_require_finite=True,
            sim_require_nnan=True,
            nc=nc,
        )
        return outs[0]

    devices = jax.devices()[:N_CORES]
    mesh = Mesh(np.asarray(devices), ("core",))
    sharded = jax.jit(
        shard_map(
            _body,
            mesh=mesh,
            in_specs=(PartitionSpec("core"), PartitionSpec("core")),
            out_specs=PartitionSpec("core"),
            check_rep=False,
        ),
        donate_argnums=(1,),
        keep_unused=True,
    )

    def run(qkv_full: np.ndarray) -> np.ndarray:
        zeros = np.zeros((N_CORES * B_CORE, SEQ, NUM_HEAD, HEAD_DIM), np.float32)
        out = sharded(qkv_full, zeros)
        return np.asarray(out)

    run.sharded = sharded
    run.mesh = mesh
    run.out_shape = (N_CORES * B_CORE, SEQ, NUM_HEAD, HEAD_DIM)
    return run


def _get_runner(L: int, repeat: int = 1):
    key = (L, repeat)
    if key not in _RUNNER_CACHE:
        _RUNNER_CACHE[key] = _make_runner(L, repeat)
    return _RUNNER_CACHE[key]


def _run(qkv: np.ndarray, kv_seq_len, trace: bool = False):
    L = int(kv_seq_len)
    L = max(1, min(SEQ, L))
    nc = _get_program(L)
    qkv = np.ascontiguousarray(np.asarray(qkv, dtype=np.float32))
    in_maps = [
        {"qkv": qkv[i * B_CORE : (i + 1) * B_CORE]} for i in range(N_CORES)
    ]
    res = run_bass_kernel_spmd(nc, in_maps, list(range(N_CORES)), trace=trace)
    outs = [res.results[i]["out"] for i in range(N_CORES)]
    full = np.concatenate(outs, axis=0).astype(np.float32)
    return full, res


def kernel(qkv: np.ndarray, kv_seq_len) -> np.ndarray:
    L = max(1, min(SEQ, int(kv_seq_len)))
    qkv = np.ascontiguousarray(np.asarray(qkv, dtype=np.float32))
    return _get_runner(L)(qkv)
